# revision 1
# baseline (speedup 1.0000x reference)
"""Trainium2 Bass kernel for causal GQA attention (nn_Attention_83090437308676).

Full shapes: x [4096, 2048], 16 Q heads / 4 KV heads, d_head=128, fp32, causal,
rotary (interleaved pairs, rotary_dim=128), out = attn @ W_O + b_O.

Sharding: tensor-parallel over heads. Core c computes Q-heads {2c, 2c+1} and
KV-head c//2 (duplicated across the pair of cores sharing it), produces the
partial output z_h @ W_O_h summed over its 2 heads; the host sums the 8
partials and adds b_O.

Device-side layout trick: all matmuls contract on the partition axis, so x is
fed pre-transposed (xT [d_model, seq]) and Q/K are produced directly in
"T" layout [d_head, seq]. Scores are computed k-major (ST [k, q]) so exp(ST)
tiles serve directly as matmul operands for both the PV product (V stationary)
and the softmax denominator (all-ones stationary -> row-broadcast denominators
in PSUM), with zero on-chip transposes of the attention pattern. Rotary is
applied in a permuted head layout (even dims first) so pair elements sit in
partition halves; W_Q/W_K/b_Q/b_K are pre-permuted on the host to match.
"""

import numpy as np

SEQ = 4096
D_MODEL = 2048
D_HEAD = 128
N_HEADS = 16
N_KV = 4
N_CORES = 8
ROTARY_BASE = 10000.0
ATTN_SCALE = 11.313708498984761  # sqrt(d_head)

P = 128  # partitions
FD = 512  # matmul moving free dim / chunk width


def build_bass(seq=SEQ, d_model=D_MODEL, heads_per_core=2):
    """Emit the per-core Tile kernel. Same program for all cores (SPMD);
    per-core tensors differ only in data."""
    from contextlib import ExitStack

    import concourse.mybir as mybir
    import concourse.tile as tile
    from concourse import bacc
    from concourse.bass import ds

    f32 = mybir.dt.float32
    f32r = mybir.dt.float32r
    AF = mybir.ActivationFunctionType
    OP = mybir.AluOpType

    H = heads_per_core
    DM_TILES = d_model // P      # contraction tiles for projections
    QC = seq // FD               # 512-wide seq chunks
    MC = d_model // FD           # 512-wide output-model chunks

    nc = bacc.Bacc("TRN2", target_bir_lowering=False, debug=False,
                   num_devices=N_CORES)

    xT = nc.dram_tensor("xT", (d_model, seq), f32r, kind="ExternalInput").ap()
    wq = nc.dram_tensor("wq", (H, d_model, D_HEAD), f32r, kind="ExternalInput").ap()
    wk = nc.dram_tensor("wk", (d_model, D_HEAD), f32r, kind="ExternalInput").ap()
    wv = nc.dram_tensor("wv", (d_model, D_HEAD), f32r, kind="ExternalInput").ap()
    wo = nc.dram_tensor("wo", (H, D_HEAD, d_model), f32r, kind="ExternalInput").ap()
    bq = nc.dram_tensor("bq", (64, H, 2), f32, kind="ExternalInput").ap()
    bk = nc.dram_tensor("bk", (64, 2), f32, kind="ExternalInput").ap()
    bv = nc.dram_tensor("bv", (P, 1), f32, kind="ExternalInput").ap()
    cos2 = nc.dram_tensor("cos2", (64, seq), f32, kind="ExternalInput").ap()
    sin2 = nc.dram_tensor("sin2", (64, seq), f32, kind="ExternalInput").ap()
    ident = nc.dram_tensor("ident", (P, P), f32, kind="ExternalInput").ap()
    maskm = nc.dram_tensor("maskm", (P, P), f32r, kind="ExternalInput").ap()
    onesd = nc.dram_tensor("onesd", (P, P), f32r, kind="ExternalInput").ap()
    out = nc.dram_tensor("out", (seq, d_model), f32, kind="ExternalOutput").ap()

    with tile.TileContext(nc) as tc, ExitStack() as ctx:
        const = ctx.enter_context(tc.tile_pool(name="const", bufs=1))
        persist = ctx.enter_context(tc.tile_pool(name="persist", bufs=1))
        xt_pool = ctx.enter_context(tc.tile_pool(name="xt", bufs=17))
        qt_pool = ctx.enter_context(tc.tile_pool(name="qt", bufs=2))
        e_pool = ctx.enter_context(tc.tile_pool(name="e", bufs=4))
        wk_pool = ctx.enter_context(tc.tile_pool(name="wk", bufs=2))
        ps = ctx.enter_context(tc.tile_pool(name="ps", bufs=8, space="PSUM"))

        # ---- constants / weights resident in SBUF ----
        # Weight-chunk DMAs are interleaved with the first chunk's xt loads
        # (inside phase1(0)) so the first projection matmuls start ~2us in.
        wq_sb = const.tile([P, H, DM_TILES, D_HEAD], f32r, tag="wq")
        wk_sb = const.tile([P, DM_TILES, D_HEAD], f32r, tag="wk")
        wv_sb = const.tile([P, DM_TILES, D_HEAD], f32r, tag="wv")
        wq_r = wq.rearrange("h (t p) d -> p h t d", p=P)
        wk_r = wk.rearrange("(t p) d -> p t d", p=P)
        wv_r = wv.rearrange("(t p) d -> p t d", p=P)
        id_sb = const.tile([P, P], f32, tag="id")
        nc.sync.dma_start(id_sb[:], ident)
        mask_sb = const.tile([P, P], f32r, tag="mask")
        nc.sync.dma_start(mask_sb[:], maskm)
        bq_sb = const.tile([64, H, 2], f32, tag="bq")
        nc.sync.dma_start(bq_sb[:], bq)
        bk_sb = const.tile([64, 2], f32, tag="bk")
        nc.sync.dma_start(bk_sb[:], bk)
        bv_sb = const.tile([P, 1], f32, tag="bv")
        nc.sync.dma_start(bv_sb[:], bv)
        ones_sb = const.tile([P, P], f32r, tag="ones")
        nc.sync.dma_start(ones_sb[:], onesd)
        cos_sb = const.tile([64, seq], f32, tag="cos")
        sin_sb = const.tile([64, seq], f32, tag="sin")
        wo_sb = const.tile([P, H, d_model], f32r, tag="wo")

        # K^T (rotated) and V (natural [k, d]) for this core's KV head.
        kt_sb = persist.tile([P, seq], f32r, tag="kt")
        v_sb = persist.tile([P, seq // P, P], f32r, tag="v")

        def rotary_evac(psum, dst, b_ap, qc):
            """dst ([P, FD] slice) = rotary(psum + bias) at positions of chunk qc.

            All DVE products run at partitions 0..63 (PSUM in0 may carry a
            different base partition; two SBUF inputs may not)."""
            sl = ds(qc * FD, FD)
            x1, x2 = psum[0:64, :], psum[64:128, :]
            b1, b2 = b_ap[:, 0:1], b_ap[:, 1:2]
            t1 = wk_pool.tile([64, FD], f32, tag="rot_t1")
            t2 = wk_pool.tile([64, FD], f32, tag="rot_t2")
            t3 = wk_pool.tile([64, FD], f32, tag="rot_t3")
            t4 = wk_pool.tile([64, FD], f32, tag="rot_t4")
            nc.vector.scalar_tensor_tensor(t1[:], x1, b1, cos_sb[:, sl],
                                           op0=OP.add, op1=OP.mult)
            nc.vector.scalar_tensor_tensor(t2[:], x2, b2, sin_sb[:, sl],
                                           op0=OP.add, op1=OP.mult)
            nc.vector.scalar_tensor_tensor(t3[:], x1, b1, sin_sb[:, sl],
                                           op0=OP.add, op1=OP.mult)
            nc.vector.scalar_tensor_tensor(t4[:], x2, b2, cos_sb[:, sl],
                                           op0=OP.add, op1=OP.mult)
            # rot1 = x1 cos - x2 sin ; rot2 = x1 sin + x2 cos
            nc.vector.tensor_sub(dst[0:64, :], t1[:], t2[:])
            nc.vector.tensor_add(dst[64:128, :], t3[:], t4[:])

        def phase1(qc):
            """Q/K/V projections for seq chunk qc (two passes over resident
            xt tiles: Q heads first, then K/V -> only 2 PSUM banks at a
            time); returns the qt tile."""
            xts = [xt_pool.tile([P, FD], f32r, tag="xt", name=f"xt_{qc}_{t}")
                   for t in range(DM_TILES)]
            qp = [ps.tile([P, FD], f32, tag="ps", name=f"qp{h}_{qc}") for h in range(H)]
            for t in range(DM_TILES):
                if qc == 0:
                    nc.sync.dma_start(wq_sb[:, :, t, :], wq_r[:, :, t, :])
                nc.sync.dma_start(xts[t][:], xT[ds(t * P, P), ds(qc * FD, FD)])
                mm = dict(start=(t == 0), stop=(t == DM_TILES - 1))
                for h in range(H):
                    nc.tensor.matmul(qp[h][:], wq_sb[:, h, t, :], xts[t][:], **mm)
            if qc == 0:
                nc.sync.dma_start(cos_sb[:], cos2)
                nc.sync.dma_start(sin_sb[:], sin2)
            qt = qt_pool.tile([P, H, FD], f32r, tag="qt", name=f"qt_{qc}")
            for h in range(H):
                rotary_evac(qp[h], qt[:, h, :], bq_sb[:, h, :], qc)

            kp = ps.tile([P, FD], f32, tag="ps", name=f"kp_{qc}")
            vp = ps.tile([P, FD], f32, tag="ps", name=f"vp_{qc}")
            for t in range(DM_TILES):
                if qc == 0:
                    nc.sync.dma_start(wk_sb[:, t, :], wk_r[:, t, :])
                    nc.sync.dma_start(wv_sb[:, t, :], wv_r[:, t, :])
                mm = dict(start=(t == 0), stop=(t == DM_TILES - 1))
                nc.tensor.matmul(kp[:], wk_sb[:, t, :], xts[t][:], **mm)
                nc.tensor.matmul(vp[:], wv_sb[:, t, :], xts[t][:], **mm)
            if qc == 0:
                nc.sync.dma_start(wo_sb[:], wo.rearrange("h p m -> p h m"))
            rotary_evac(kp, kt_sb[:, ds(qc * FD, FD)], bk_sb, qc)
            # V: bias add then transpose to natural [k, d] layout
            vt = wk_pool.tile([P, FD], f32, tag="vt")
            nc.scalar.activation(vt[:], vp[:], AF.Identity, bias=bv_sb[:, 0:1])
            for j in range(FD // P):
                tp = ps.tile([P, P], f32, tag="ps", name=f"tp_{qc}_{j}")
                nc.tensor.transpose(tp[:], vt[:, ds(j * P, P)], id_sb[:])
                nc.scalar.copy(v_sb[:, qc * (FD // P) + j, :], tp[:])
            return qt

        def attention(qc, qt):
            """Causal attention for q chunk qc; returns per-head normalized z^T."""
            ztn = []
            for h in range(H):
                zt = ps.tile([P, FD], f32, tag="ps", name=f"zt_{h}_{qc}")
                den = ps.tile([P, FD], f32, tag="ps", name=f"den_{h}_{qc}")
                kt_max = 4 * qc + 3
                for kt in range(kt_max + 1):
                    o = max(0, kt * P - qc * FD)
                    n = FD - o
                    st = ps.tile([P, FD], f32, tag="ps", name=f"st_{h}_{qc}_{kt}")
                    nc.tensor.matmul(st[:, o:FD], kt_sb[:, ds(kt * P, P)],
                                     qt[:, h, o:FD], start=True, stop=True)
                    e = e_pool.tile([P, FD], f32r, tag="e", name=f"e_{h}_{qc}_{kt}")
                    nc.scalar.activation(e[:, o:FD], st[:, o:FD], AF.Exp,
                                         scale=1.0 / ATTN_SCALE)
                    if kt >= 4 * qc:  # diagonal 128-block: causal mask inside
                        nc.vector.tensor_mul(e[:, o:o + P], e[:, o:o + P], mask_sb[:])
                    acc = dict(start=(kt == 0), stop=(kt == kt_max))
                    nc.tensor.matmul(zt[:, o:FD], v_sb[:, kt, :], e[:, o:FD], **acc)
                    nc.tensor.matmul(den[0:1, o:FD], ones_sb[:, 0:1], e[:, o:FD], **acc)
                # reciprocal of one denominator row, broadcast via K=1 matmul
                rf = wk_pool.tile([1, FD], f32, tag="rf", bufs=1, name=f"rf_{h}_{qc}")
                nc.vector.reciprocal_approx_fast(rf[:], den[0:1, :])
                rr = wk_pool.tile([1, FD], f32r, tag="rr", bufs=1, name=f"rr_{h}_{qc}")
                nc.vector.tensor_scalar_mul(rr[:], rf[:], 1.0)
                # broadcast 1/den into the (already-read) den bank: saves a
                # PSUM slot so the other head's matmuls run during this chain
                nc.tensor.matmul(den[:], ones_sb[0:1, :], rr[:],
                                 start=True, stop=True)
                rden = wk_pool.tile([P, FD], f32, tag="rden", name=f"rd_{h}_{qc}")
                nc.vector.tensor_copy(rden[:], den[:])
                z = wk_pool.tile([P, FD], f32r, tag="ztn", bufs=3, name=f"z_{h}_{qc}")
                nc.vector.tensor_mul(z[:], zt[:], rden[:])
                ztn.append(z)
            return ztn

        def outproj(qc, ztn):
            for sub in range(FD // P):
                for mc in range(MC):
                    op_ps = ps.tile([P, FD], f32, tag="ps", name=f"op_{qc}_{sub}_{mc}")
                    for h in range(H):
                        nc.tensor.matmul(op_ps[:], ztn[h][:, ds(sub * P, P)],
                                         wo_sb[:, h, ds(mc * FD, FD)],
                                         start=(h == 0), stop=(h == H - 1))
                    ot = wk_pool.tile([P, FD], f32, tag="ot", bufs=2,
                                      name=f"ot_{qc}_{sub}_{mc}")
                    nc.scalar.copy(ot[:], op_ps[:])
                    nc.sync.dma_start(out[ds(qc * FD + sub * P, P), ds(mc * FD, FD)],
                                      ot[:])

        # Software pipeline: projections for chunk qc+1 are emitted before
        # attention of chunk qc so the PE always has runnable matmuls while
        # attention waits on softmax chains.
        qts = {0: phase1(0)}
        for qc in range(QC):
            if qc + 1 < QC:
                qts[qc + 1] = phase1(qc + 1)
            ztn = attention(qc, qts.pop(qc))
            outproj(qc, ztn)
    nc.compile()
    return nc


_PERM = None


def _perm():
    global _PERM
    if _PERM is None:
        _PERM = np.concatenate([np.arange(0, D_HEAD, 2), np.arange(1, D_HEAD, 2)])
    return _PERM


def host_inputs(x, W_Q, W_K, W_V, W_O, b_Q, b_K, b_V, core,
                heads_per_core=2):
    """Build the per-core input map (numpy, named as in build_bass)."""
    seq = x.shape[0]
    perm = _perm()
    h0 = core * heads_per_core
    kv = h0 // (N_HEADS // N_KV)
    pairs = D_HEAD // 2
    freqs = 1.0 / ROTARY_BASE ** (np.arange(pairs, dtype=np.float64) / pairs)
    ang = np.outer(np.arange(seq), freqs)  # [seq, 64]
    cos = np.cos(ang).T.astype(np.float32)  # [64, seq]
    sin = np.sin(ang).T.astype(np.float32)
    return {
        "xT": np.ascontiguousarray(x.T),
        "wq": np.ascontiguousarray(W_Q[h0:h0 + heads_per_core][:, :, perm]),
        "wk": np.ascontiguousarray(W_K[kv][:, perm]),
        "wv": np.ascontiguousarray(W_V[kv]),
        "wo": np.ascontiguousarray(W_O[h0:h0 + heads_per_core]),
        "bq": np.ascontiguousarray(
            b_Q[h0:h0 + heads_per_core][:, perm]
            .reshape(heads_per_core, 2, 64).transpose(2, 0, 1)),
        "bk": np.ascontiguousarray(b_K[kv][perm].reshape(2, 64).T),
        "bv": np.ascontiguousarray(b_V[kv][:, None]),
        "cos2": cos,
        "sin2": sin,
        "ident": np.eye(P, dtype=np.float32),
        "maskm": np.triu(np.ones((P, P), dtype=np.float32)),
        "onesd": np.ones((P, P), dtype=np.float32),
    }


_NC_CACHE = {}


def kernel(x, W_Q, W_K, W_V, W_O, b_Q, b_K, b_V, b_O):
    import sys
    if "/opt/trn_rl_repo" not in sys.path:
        sys.path.insert(0, "/opt/trn_rl_repo")
    from concourse import bass_utils

    x = np.asarray(x, dtype=np.float32)
    key = (x.shape[0], x.shape[1])
    if key not in _NC_CACHE:
        _NC_CACHE[key] = build_bass(seq=x.shape[0], d_model=x.shape[1])
    nc = _NC_CACHE[key]

    in_maps = [
        host_inputs(x, np.asarray(W_Q, np.float32), np.asarray(W_K, np.float32),
                    np.asarray(W_V, np.float32), np.asarray(W_O, np.float32),
                    np.asarray(b_Q, np.float32), np.asarray(b_K, np.float32),
                    np.asarray(b_V, np.float32), core)
        for core in range(N_CORES)
    ]
    res = bass_utils.run_bass_kernel_spmd(nc, in_maps, core_ids=list(range(N_CORES)))
    total = np.zeros((x.shape[0], x.shape[1]), dtype=np.float32)
    for r in res.results:
        total += r["out"]
    total += np.asarray(b_O, np.float32)[None, :]
    return total



# revision 9
# speedup vs baseline: 1.0950x; 1.0950x over previous
"""Trainium2 Bass kernel for causal GQA attention (nn_Attention_83090437308676).

Full shapes: x [4096, 2048], 16 Q heads / 4 KV heads, d_head=128, fp32, causal,
rotary (interleaved pairs, rotary_dim=128), out = attn @ W_O + b_O.

Sharding: tensor-parallel over heads. Core c computes Q-heads {2c, 2c+1} and
KV-head c//2 (duplicated across the pair of cores sharing it), produces the
partial output z_h @ W_O_h summed over its 2 heads; the host sums the 8
partials (bf16 on the wire, fp32 accumulate) and adds b_O.

v2 design notes (vs the fp32r baseline at 624us):
- All matmul operands are bf16 (PSUM accumulation stays fp32). Halves HBM
  traffic and SBUF pressure; diagonal partial tiles no longer pay the f32r
  narrow-moving-dim penalty.
- V is projected directly into its natural [k, d] layout (stationary = xT
  block, moving = W_V tile), eliminating the PE transposes + Act copies.
- Attention inner loop is software-pipelined with a full-iteration skew:
  per kt, the st matmuls of both heads are emitted first, then the PV/den
  matmuls of the previous kt. The PE never waits on the Act-engine exp.
- Softmax reciprocal broadcast stays on the PE (baseline scheme): the
  Pool engine's partition_broadcast faulted the exec unit on hardware.
- Per-chunk emission order is attention(qc) -> proj(qc+1) -> outproj(qc) so
  the projection matmuls hide the softmax-normalization tail and outproj
  only runs once the normalized z tiles are long ready.
- Output partials are evacuated PSUM->SBUF as bf16 (alternating Act/DVE)
  and DMA'd as bf16, halving the 32MB output write.
"""

import numpy as np

SEQ = 4096
D_MODEL = 2048
D_HEAD = 128
N_HEADS = 16
N_KV = 4
N_CORES = 8
ROTARY_BASE = 10000.0
ATTN_SCALE = 11.313708498984761  # sqrt(d_head)

P = 128  # partitions
FD = 512  # matmul moving free dim / chunk width


def build_bass(seq=SEQ, d_model=D_MODEL, heads_per_core=2):
    """Emit the per-core Tile kernel. Same program for all cores (SPMD);
    per-core tensors differ only in data."""
    from contextlib import ExitStack

    import concourse.mybir as mybir
    import concourse.tile as tile
    from concourse import bacc
    from concourse.bass import ds

    f32 = mybir.dt.float32
    f32r = mybir.dt.float32r
    bf16 = mybir.dt.bfloat16
    AF = mybir.ActivationFunctionType
    OP = mybir.AluOpType

    H = heads_per_core
    DM_TILES = d_model // P      # contraction tiles for projections
    QC = seq // FD               # 512-wide seq chunks
    MC = d_model // FD           # 512-wide output-model chunks
    KB = FD // P                 # 128-wide k blocks per chunk

    nc = bacc.Bacc("TRN2", target_bir_lowering=False, debug=False,
                   num_devices=N_CORES)

    xT = nc.dram_tensor("xT", (d_model, seq), bf16, kind="ExternalInput").ap()
    wq = nc.dram_tensor("wq", (H, d_model, D_HEAD), bf16, kind="ExternalInput").ap()
    wk = nc.dram_tensor("wk", (d_model, D_HEAD), bf16, kind="ExternalInput").ap()
    wv = nc.dram_tensor("wv", (d_model, D_HEAD), bf16, kind="ExternalInput").ap()
    wo = nc.dram_tensor("wo", (H, D_HEAD, d_model), bf16, kind="ExternalInput").ap()
    bq = nc.dram_tensor("bq", (64, H, 2), f32, kind="ExternalInput").ap()
    bk = nc.dram_tensor("bk", (64, 2), f32, kind="ExternalInput").ap()
    bvb = nc.dram_tensor("bvb", (P, FD), f32, kind="ExternalInput").ap()
    cos2 = nc.dram_tensor("cos2", (64, seq), f32, kind="ExternalInput").ap()
    sin2 = nc.dram_tensor("sin2", (64, seq), f32, kind="ExternalInput").ap()
    maskm = nc.dram_tensor("maskm", (P, P), bf16, kind="ExternalInput").ap()
    onesd = nc.dram_tensor("onesd", (P, 1), bf16, kind="ExternalInput").ap()
    onesr = nc.dram_tensor("onesr", (1, P), f32r, kind="ExternalInput").ap()
    out = nc.dram_tensor("out", (seq, d_model), bf16, kind="ExternalOutput").ap()

    with tile.TileContext(nc) as tc, ExitStack() as ctx:
        const = ctx.enter_context(tc.tile_pool(name="const", bufs=1))
        persist = ctx.enter_context(tc.tile_pool(name="persist", bufs=1))
        xt_pool = ctx.enter_context(tc.tile_pool(name="xt", bufs=32))
        qt_pool = ctx.enter_context(tc.tile_pool(name="qt", bufs=2))
        e_pool = ctx.enter_context(tc.tile_pool(name="e", bufs=6))
        sc_pool = ctx.enter_context(tc.tile_pool(name="sc", bufs=2))
        ps = ctx.enter_context(tc.tile_pool(name="ps", bufs=8, space="PSUM"))

        # ---- constants / weights resident in SBUF ----
        wq_sb = const.tile([P, H, DM_TILES, D_HEAD], bf16, tag="wq")
        wk_sb = const.tile([P, DM_TILES, D_HEAD], bf16, tag="wk")
        wv_sb = const.tile([P, DM_TILES, D_HEAD], bf16, tag="wv")
        wq_r = wq.rearrange("h (t p) d -> p h t d", p=P)
        wk_r = wk.rearrange("(t p) d -> p t d", p=P)
        wv_r = wv.rearrange("(t p) d -> p t d", p=P)
        mask_sb = const.tile([P, P], bf16, tag="mask")
        nc.sync.dma_start(mask_sb[:], maskm)
        bq_sb = const.tile([64, H, 2], f32, tag="bq")
        nc.sync.dma_start(bq_sb[:], bq)
        bk_sb = const.tile([64, 2], f32, tag="bk")
        nc.sync.dma_start(bk_sb[:], bk)
        bvb_sb = const.tile([P, FD], f32, tag="bvb")
        nc.sync.dma_start(bvb_sb[:], bvb)
        ones_sb = const.tile([P, 1], bf16, tag="ones")
        nc.sync.dma_start(ones_sb[:], onesd)
        onesr_sb = const.tile([1, P], f32r, tag="onesr")
        nc.sync.dma_start(onesr_sb[:], onesr)
        cos_sb = const.tile([64, seq], f32, tag="cos")
        sin_sb = const.tile([64, seq], f32, tag="sin")
        wo_sb = const.tile([P, H, d_model], bf16, tag="wo")

        # K^T (rotated) and V (natural [k, d]) for this core's KV head.
        kt_sb = persist.tile([P, seq], bf16, tag="kt")
        v_sb = persist.tile([P, seq // P, P], bf16, tag="v")

        xts = {}  # chunk -> list of resident xT tiles

        def prefetch_x(qc):
            tiles = [xt_pool.tile([P, FD], bf16, tag="xt", name=f"xt_{qc}_{t}")
                     for t in range(DM_TILES)]
            for t in range(DM_TILES):
                nc.sync.dma_start(tiles[t][:], xT[ds(t * P, P), ds(qc * FD, FD)])
            xts[qc] = tiles

        def rotary_evac(psum, dst, b_ap, qc):
            """dst ([P, FD] slice, bf16) = rotary(psum + bias) at positions of
            chunk qc. All DVE products run at partitions 0..63 (PSUM in0 may
            carry a different base partition; two SBUF inputs may not)."""
            sl = ds(qc * FD, FD)
            x1, x2 = psum[0:64, :], psum[64:128, :]
            b1, b2 = b_ap[:, 0:1], b_ap[:, 1:2]
            t1 = sc_pool.tile([64, FD], f32, tag="rot_t1")
            t2 = sc_pool.tile([64, FD], f32, tag="rot_t2")
            t3 = sc_pool.tile([64, FD], f32, tag="rot_t3")
            t4 = sc_pool.tile([64, FD], f32, tag="rot_t4")
            nc.vector.scalar_tensor_tensor(t1[:], x1, b1, cos_sb[:, sl],
                                           op0=OP.add, op1=OP.mult)
            nc.vector.scalar_tensor_tensor(t2[:], x2, b2, sin_sb[:, sl],
                                           op0=OP.add, op1=OP.mult)
            nc.vector.scalar_tensor_tensor(t3[:], x1, b1, sin_sb[:, sl],
                                           op0=OP.add, op1=OP.mult)
            nc.vector.scalar_tensor_tensor(t4[:], x2, b2, cos_sb[:, sl],
                                           op0=OP.add, op1=OP.mult)
            # rot1 = x1 cos - x2 sin ; rot2 = x1 sin + x2 cos
            nc.vector.tensor_sub(dst[0:64, :], t1[:], t2[:])
            nc.vector.tensor_add(dst[64:128, :], t3[:], t4[:])

        def proj(qc):
            """Q/K/V projections for seq chunk qc. K first (its rotary
            unblocks the next attention chunk's diagonal), then Q heads,
            then V directly in natural [k, d] layout."""
            tiles = xts.pop(qc)
            if qc == 0:
                for t in range(DM_TILES):
                    nc.sync.dma_start(wk_sb[:, t, :], wk_r[:, t, :])
            kp = ps.tile([P, FD], f32, tag="ps", name=f"kp_{qc}")
            for t in range(DM_TILES):
                nc.tensor.matmul(kp[:], wk_sb[:, t, :], tiles[t][:],
                                 start=(t == 0), stop=(t == DM_TILES - 1))
            if qc == 0:
                nc.sync.dma_start(cos_sb[:], cos2)
                nc.sync.dma_start(sin_sb[:], sin2)
                for t in range(DM_TILES):
                    nc.sync.dma_start(wq_sb[:, :, t, :], wq_r[:, :, t, :])
            rotary_evac(kp, kt_sb[:, ds(qc * FD, FD)], bk_sb, qc)

            qt = qt_pool.tile([P, H, FD], bf16, tag="qt", name=f"qt_{qc}")
            qp = [ps.tile([P, FD], f32, tag="ps", name=f"qp{h}_{qc}")
                  for h in range(H)]
            for t in range(DM_TILES):
                mm = dict(start=(t == 0), stop=(t == DM_TILES - 1))
                for h in range(H):
                    nc.tensor.matmul(qp[h][:], wq_sb[:, h, t, :], tiles[t][:], **mm)
            if qc == 0:
                for t in range(DM_TILES):
                    nc.sync.dma_start(wv_sb[:, t, :], wv_r[:, t, :])
            for h in range(H):
                rotary_evac(qp[h], qt[:, h, :], bq_sb[:, h, :], qc)

            # V: natural [k, d] layout; stationary = xT block, moving = W_V.
            vp = ps.tile([P, FD], f32, tag="ps", name=f"vp_{qc}")
            for kb in range(KB):
                for t in range(DM_TILES):
                    nc.tensor.matmul(vp[:, ds(kb * P, P)],
                                     tiles[t][:, ds(kb * P, P)], wv_sb[:, t, :],
                                     start=(t == 0), stop=(t == DM_TILES - 1))
            if qc == 0:
                nc.sync.dma_start(wo_sb[:], wo.rearrange("h p m -> p h m"))
            # bias add (b_V pre-broadcast from host) + bf16 downcast
            nc.vector.scalar_tensor_tensor(
                v_sb[:, ds(qc * KB, KB), :].rearrange("p a b -> p (a b)"),
                vp[:], 1.0, bvb_sb[:], op0=OP.mult, op1=OP.add)
            return qt

        def attention(qc, qt):
            """Causal attention for q chunk qc; returns per-head normalized
            z^T (bf16). Inner loop is skewed one full kt iteration: st
            matmuls of both heads first, then PV/den of the previous kt."""
            KT = 4 * qc + 4
            zt = [ps.tile([P, FD], f32, tag="ps", name=f"zt{h}_{qc}")
                  for h in range(H)]
            den = [ps.tile([P, FD], f32, tag="ps", name=f"den{h}_{qc}")
                   for h in range(H)]
            pend = []
            for kt in range(KT):
                o = max(0, kt * P - qc * FD)
                cur = []
                for h in range(H):
                    st = ps.tile([P, FD], f32, tag="ps", name=f"st{h}_{qc}_{kt}")
                    nc.tensor.matmul(st[:, o:FD], kt_sb[:, ds(kt * P, P)],
                                     qt[:, h, o:FD], start=True, stop=True)
                    e = e_pool.tile([P, FD], bf16, tag="e", name=f"e{h}_{qc}_{kt}")
                    nc.scalar.activation(e[:, o:FD], st[:, o:FD], AF.Exp,
                                         scale=1.0 / ATTN_SCALE)
                    if kt >= 4 * qc:  # diagonal 128-block: causal mask inside
                        nc.vector.tensor_mul(e[:, o:o + P], e[:, o:o + P],
                                             mask_sb[:])
                    cur.append((h, kt, e, o))
                for h, pkt, e, po in pend:
                    acc = dict(start=(pkt == 0), stop=(pkt == KT - 1))
                    nc.tensor.matmul(zt[h][:, po:FD], v_sb[:, pkt, :],
                                     e[:, po:FD], **acc)
                    nc.tensor.matmul(den[h][0:1, po:FD], ones_sb[:, 0:1],
                                     e[:, po:FD], **acc)
                pend = cur
            for h, pkt, e, po in pend:
                acc = dict(start=(pkt == 0), stop=(pkt == KT - 1))
                nc.tensor.matmul(zt[h][:, po:FD], v_sb[:, pkt, :],
                                 e[:, po:FD], **acc)
                nc.tensor.matmul(den[h][0:1, po:FD], ones_sb[:, 0:1],
                                 e[:, po:FD], **acc)
            # normalization: reciprocal of one denominator row, broadcast via
            # K=1 matmul into the (already-read) den bank, z = zt * (1/den)
            ztn = []
            for h in range(H):
                rf = sc_pool.tile([1, FD], f32, tag="rf", name=f"rf{h}_{qc}")
                nc.vector.reciprocal_approx_fast(rf[:], den[h][0:1, :])
                rr = sc_pool.tile([1, FD], f32r, tag="rr", name=f"rr{h}_{qc}")
                nc.vector.tensor_scalar_mul(rr[:], rf[:], 1.0)
                nc.tensor.matmul(den[h][:], onesr_sb[:], rr[:],
                                 start=True, stop=True)
                rden = sc_pool.tile([P, FD], f32, tag="rden", name=f"rd{h}_{qc}")
                nc.vector.tensor_copy(rden[:], den[h][:])
                z = sc_pool.tile([P, FD], bf16, tag="z", bufs=4, name=f"z{h}_{qc}")
                nc.vector.tensor_mul(z[:], zt[h][:], rden[:])
                ztn.append(z)
            return ztn

        def outproj(qc, ztn):
            for sub in range(KB):
                for mc in range(MC):
                    op_ps = ps.tile([P, FD], f32, tag="ps",
                                    name=f"op_{qc}_{sub}_{mc}")
                    for h in range(H):
                        nc.tensor.matmul(op_ps[:], ztn[h][:, ds(sub * P, P)],
                                         wo_sb[:, h, ds(mc * FD, FD)],
                                         start=(h == 0), stop=(h == H - 1))
                    ot = sc_pool.tile([P, FD], bf16, tag="ot", bufs=3,
                                      name=f"ot_{qc}_{sub}_{mc}")
                    if (sub * MC + mc) % 2 == 0:
                        nc.scalar.copy(ot[:], op_ps[:])
                    else:
                        nc.vector.tensor_copy(ot[:], op_ps[:])
                    nc.sync.dma_start(out[ds(qc * FD + sub * P, P), ds(mc * FD, FD)],
                                      ot[:])

        prefetch_x(0)
        qts = {0: proj(0)}
        for qc in range(QC):
            if qc + 1 < QC:
                prefetch_x(qc + 1)
            ztn = attention(qc, qts.pop(qc))
            if qc + 1 < QC:
                qts[qc + 1] = proj(qc + 1)
            outproj(qc, ztn)
    nc.compile()
    return nc


_PERM = None


def _perm():
    global _PERM
    if _PERM is None:
        _PERM = np.concatenate([np.arange(0, D_HEAD, 2), np.arange(1, D_HEAD, 2)])
    return _PERM


def host_inputs(x, W_Q, W_K, W_V, W_O, b_Q, b_K, b_V, core,
                heads_per_core=2):
    """Build the per-core input map (numpy, named as in build_bass)."""
    import ml_dtypes

    bf16 = ml_dtypes.bfloat16
    seq = x.shape[0]
    perm = _perm()
    h0 = core * heads_per_core
    kv = h0 // (N_HEADS // N_KV)
    pairs = D_HEAD // 2
    freqs = 1.0 / ROTARY_BASE ** (np.arange(pairs, dtype=np.float64) / pairs)
    ang = np.outer(np.arange(seq), freqs)  # [seq, 64]
    cos = np.cos(ang).T.astype(np.float32)  # [64, seq]
    sin = np.sin(ang).T.astype(np.float32)
    return {
        "xT": np.ascontiguousarray(np.asarray(x).T.astype(bf16)),
        "wq": np.ascontiguousarray(
            W_Q[h0:h0 + heads_per_core][:, :, perm].astype(bf16)),
        "wk": np.ascontiguousarray(W_K[kv][:, perm].astype(bf16)),
        "wv": np.ascontiguousarray(W_V[kv].astype(bf16)),
        "wo": np.ascontiguousarray(W_O[h0:h0 + heads_per_core].astype(bf16)),
        "bq": np.ascontiguousarray(
            b_Q[h0:h0 + heads_per_core][:, perm]
            .reshape(heads_per_core, 2, 64).transpose(2, 0, 1)
            .astype(np.float32)),
        "bk": np.ascontiguousarray(b_K[kv][perm].reshape(2, 64).T
                                   .astype(np.float32)),
        "bvb": np.ascontiguousarray(
            np.tile(np.asarray(b_V[kv], np.float32)[None, :], (P, FD // D_HEAD))),
        "cos2": cos,
        "sin2": sin,
        "maskm": np.triu(np.ones((P, P), dtype=np.float32)).astype(bf16),
        "onesd": np.ones((P, 1), dtype=np.float32).astype(bf16),
        "onesr": np.ones((1, P), dtype=np.float32),
    }


_NC_CACHE = {}


def kernel(x, W_Q, W_K, W_V, W_O, b_Q, b_K, b_V, b_O):
    import sys
    if "/opt/trn_rl_repo" not in sys.path:
        sys.path.insert(0, "/opt/trn_rl_repo")
    from concourse import bass_utils

    x = np.asarray(x, dtype=np.float32)
    key = (x.shape[0], x.shape[1])
    if key not in _NC_CACHE:
        _NC_CACHE[key] = build_bass(seq=x.shape[0], d_model=x.shape[1])
    nc = _NC_CACHE[key]

    in_maps = [
        host_inputs(x, np.asarray(W_Q, np.float32), np.asarray(W_K, np.float32),
                    np.asarray(W_V, np.float32), np.asarray(W_O, np.float32),
                    np.asarray(b_Q, np.float32), np.asarray(b_K, np.float32),
                    np.asarray(b_V, np.float32), core)
        for core in range(N_CORES)
    ]
    res = bass_utils.run_bass_kernel_spmd(nc, in_maps, core_ids=list(range(N_CORES)))
    total = np.zeros((x.shape[0], x.shape[1]), dtype=np.float32)
    for r in res.results:
        total += np.asarray(r["out"], dtype=np.float32)
    total += np.asarray(b_O, np.float32)[None, :]
    return total


# revision 18
# speedup vs baseline: 1.1076x; 1.0115x over previous
"""Trainium2 Bass kernel for causal GQA attention (nn_Attention_83090437308676).

Full shapes: x [4096, 2048], 16 Q heads / 4 KV heads, d_head=128, fp32, causal,
rotary (interleaved pairs, rotary_dim=128), out = attn @ W_O + b_O.

Sharding: tensor-parallel over heads. Core c computes Q-heads {2c, 2c+1} and
KV-head c//2 (duplicated across the pair of cores sharing it), produces the
partial output z_h @ W_O_h summed over its 2 heads; the host sums the 8
partials (bf16 on the wire, fp32 accumulate) and adds b_O.

v2 design notes (vs the fp32r baseline at 624us):
- All matmul operands are bf16 (PSUM accumulation stays fp32). Halves HBM
  traffic and SBUF pressure; diagonal partial tiles no longer pay the f32r
  narrow-moving-dim penalty.
- Attention inner loop is software-pipelined with a full-iteration skew:
  per kt, the st matmuls of both heads are emitted first, then the PV/den
  matmuls of the previous kt. The PE never waits on the Act-engine exp.
- Softmax reciprocal broadcast stays on the PE (baseline scheme): the
  Pool engine's partition_broadcast faulted the exec unit on hardware. The
  broadcast matmuls are deferred into the next chunk's projection block so
  the PE never waits on the DVE reciprocal chain.
- Per-chunk emission order is attention(qc) -> proj(qc+1) -> outproj(qc) so
  the projection matmuls hide the softmax-normalization tail and outproj
  only runs once the normalized z tiles are long ready.
- The Act engine runs (almost) only the exp chain: output-projection
  evacuation lives on the DVE, the 1/den broadcast copy on Act. Keeping
  exp unqueued matters: zt/den matmuls wait on it one iteration later.
- Startup DMAs are emitted just-in-time per tile (wk/xt pairs, then wq,
  then cos/sin/wv/wo) so the first kp matmul waits on ~2 DMAs, not ~48.
- Output partials are evacuated PSUM->SBUF as bf16 on the DVE and DMA'd
  as bf16, halving the 32MB output write.
"""

import numpy as np

SEQ = 4096
D_MODEL = 2048
D_HEAD = 128
N_HEADS = 16
N_KV = 4
N_CORES = 8
ROTARY_BASE = 10000.0
ATTN_SCALE = 11.313708498984761  # sqrt(d_head)

P = 128  # partitions
FD = 512  # matmul moving free dim / chunk width


def build_bass(seq=SEQ, d_model=D_MODEL, heads_per_core=2):
    """Emit the per-core Tile kernel. Same program for all cores (SPMD);
    per-core tensors differ only in data."""
    from contextlib import ExitStack

    import concourse.mybir as mybir
    import concourse.tile as tile
    from concourse import bacc
    from concourse.bass import ds

    f32 = mybir.dt.float32
    f32r = mybir.dt.float32r
    bf16 = mybir.dt.bfloat16
    AF = mybir.ActivationFunctionType
    OP = mybir.AluOpType

    H = heads_per_core
    DM_TILES = d_model // P      # contraction tiles for projections
    QC = seq // FD               # 512-wide seq chunks
    MC = d_model // FD           # 512-wide output-model chunks
    KB = FD // P                 # 128-wide k blocks per chunk

    nc = bacc.Bacc("TRN2", target_bir_lowering=False, debug=False,
                   num_devices=N_CORES)

    xT = nc.dram_tensor("xT", (d_model, seq), bf16, kind="ExternalInput").ap()
    wq = nc.dram_tensor("wq", (H, d_model, D_HEAD), bf16, kind="ExternalInput").ap()
    wk = nc.dram_tensor("wk", (d_model, D_HEAD), bf16, kind="ExternalInput").ap()
    wv = nc.dram_tensor("wv", (d_model, D_HEAD), bf16, kind="ExternalInput").ap()
    wo = nc.dram_tensor("wo", (H, D_HEAD, d_model), bf16, kind="ExternalInput").ap()
    bq = nc.dram_tensor("bq", (64, H, 2), f32, kind="ExternalInput").ap()
    bk = nc.dram_tensor("bk", (64, 2), f32, kind="ExternalInput").ap()
    bv = nc.dram_tensor("bv", (P, 1), f32, kind="ExternalInput").ap()
    ident = nc.dram_tensor("ident", (P, P), bf16, kind="ExternalInput").ap()
    cos2 = nc.dram_tensor("cos2", (64, seq), f32, kind="ExternalInput").ap()
    sin2 = nc.dram_tensor("sin2", (64, seq), f32, kind="ExternalInput").ap()
    maskm = nc.dram_tensor("maskm", (P, P), bf16, kind="ExternalInput").ap()
    onesd = nc.dram_tensor("onesd", (P, 1), bf16, kind="ExternalInput").ap()
    onesr = nc.dram_tensor("onesr", (1, P), f32r, kind="ExternalInput").ap()
    out = nc.dram_tensor("out", (seq, d_model), bf16, kind="ExternalOutput").ap()

    with tile.TileContext(nc) as tc, ExitStack() as ctx:
        const = ctx.enter_context(tc.tile_pool(name="const", bufs=1))
        persist = ctx.enter_context(tc.tile_pool(name="persist", bufs=1))
        xt_pool = ctx.enter_context(tc.tile_pool(name="xt", bufs=32))
        qt_pool = ctx.enter_context(tc.tile_pool(name="qt", bufs=2))
        e_pool = ctx.enter_context(tc.tile_pool(name="e", bufs=6))
        sc_pool = ctx.enter_context(tc.tile_pool(name="sc", bufs=2))
        ps = ctx.enter_context(tc.tile_pool(name="ps", bufs=8, space="PSUM"))

        # ---- constants / weights resident in SBUF ----
        wq_sb = const.tile([P, H, DM_TILES, D_HEAD], bf16, tag="wq")
        wk_sb = const.tile([P, DM_TILES, D_HEAD], bf16, tag="wk")
        wv_sb = const.tile([P, DM_TILES, D_HEAD], bf16, tag="wv")
        wq_r = wq.rearrange("h (t p) d -> p h t d", p=P)
        wk_r = wk.rearrange("(t p) d -> p t d", p=P)
        wv_r = wv.rearrange("(t p) d -> p t d", p=P)
        mask_sb = const.tile([P, P], bf16, tag="mask")
        nc.sync.dma_start(mask_sb[:], maskm)
        bq_sb = const.tile([64, H, 2], f32, tag="bq")
        nc.sync.dma_start(bq_sb[:], bq)
        bk_sb = const.tile([64, 2], f32, tag="bk")
        nc.sync.dma_start(bk_sb[:], bk)
        bv_sb = const.tile([P, 1], f32, tag="bv")
        nc.sync.dma_start(bv_sb[:], bv)
        id_sb = const.tile([P, P], bf16, tag="id")
        nc.sync.dma_start(id_sb[:], ident)
        ones_sb = const.tile([P, 1], bf16, tag="ones")
        nc.sync.dma_start(ones_sb[:], onesd)
        onesr_sb = const.tile([1, P], f32r, tag="onesr")
        nc.sync.dma_start(onesr_sb[:], onesr)
        cos_sb = const.tile([64, seq], f32, tag="cos")
        sin_sb = const.tile([64, seq], f32, tag="sin")
        wo_sb = const.tile([P, H, d_model], bf16, tag="wo")
        # preload the Exp activation table off the critical path
        warm = const.tile([1, 2], f32, tag="warm")
        nc.scalar.activation(warm[0:1, 0:2], bq_sb[0:1, 0, 0:2], AF.Exp)

        # K^T (rotated) and V (natural [k, d]) for this core's KV head.
        kt_sb = persist.tile([P, seq], bf16, tag="kt")
        v_sb = persist.tile([P, seq // P, P], bf16, tag="v")

        xts = {}  # chunk -> list of resident xT tiles

        def prefetch_x(qc):
            tiles = [xt_pool.tile([P, FD], bf16, tag="xt", name=f"xt_{qc}_{t}")
                     for t in range(DM_TILES)]
            for t in range(DM_TILES):
                nc.sync.dma_start(tiles[t][:], xT[ds(t * P, P), ds(qc * FD, FD)])
            xts[qc] = tiles

        def rotary_evac(psum, dst, b_ap, qc):
            """dst ([P, FD] slice, bf16) = rotary(psum + bias) at positions of
            chunk qc. All DVE products run at partitions 0..63 (PSUM in0 may
            carry a different base partition; two SBUF inputs may not)."""
            sl = ds(qc * FD, FD)
            x1, x2 = psum[0:64, :], psum[64:128, :]
            b1, b2 = b_ap[:, 0:1], b_ap[:, 1:2]
            t1 = sc_pool.tile([64, FD], f32, tag="rot_t1")
            t2 = sc_pool.tile([64, FD], f32, tag="rot_t2")
            t3 = sc_pool.tile([64, FD], f32, tag="rot_t3")
            t4 = sc_pool.tile([64, FD], f32, tag="rot_t4")
            nc.vector.scalar_tensor_tensor(t1[:], x1, b1, cos_sb[:, sl],
                                           op0=OP.add, op1=OP.mult)
            nc.vector.scalar_tensor_tensor(t2[:], x2, b2, sin_sb[:, sl],
                                           op0=OP.add, op1=OP.mult)
            nc.vector.scalar_tensor_tensor(t3[:], x1, b1, sin_sb[:, sl],
                                           op0=OP.add, op1=OP.mult)
            nc.vector.scalar_tensor_tensor(t4[:], x2, b2, cos_sb[:, sl],
                                           op0=OP.add, op1=OP.mult)
            # rot1 = x1 cos - x2 sin ; rot2 = x1 sin + x2 cos
            nc.vector.tensor_sub(dst[0:64, :], t1[:], t2[:])
            nc.vector.tensor_add(dst[64:128, :], t3[:], t4[:])

        def proj(qc, finish=None):
            """Q/K/V projections for seq chunk qc. K first (its rotary
            unblocks the next attention chunk's diagonal), then Q heads,
            then V (transposed to natural [k, d] via the PE). For qc == 0
            every weight DMA is emitted just-in-time next to its first
            consumer so the PE starts ~0.5us in. `finish` (deferred
            normalization of the previous chunk) is called after the kp
            chain, when its DVE reciprocal inputs are long done."""
            kp = ps.tile([P, FD], f32, tag="ps", name=f"kp_{qc}")
            if qc == 0:
                tiles = [xt_pool.tile([P, FD], bf16, tag="xt", name=f"xt_0_{t}")
                         for t in range(DM_TILES)]
                for t in range(DM_TILES):
                    nc.sync.dma_start(wk_sb[:, t, :], wk_r[:, t, :])
                    nc.sync.dma_start(tiles[t][:], xT[ds(t * P, P), ds(0, FD)])
                    nc.tensor.matmul(kp[:], wk_sb[:, t, :], tiles[t][:],
                                     start=(t == 0), stop=(t == DM_TILES - 1))
            else:
                tiles = xts.pop(qc)
                for t in range(DM_TILES):
                    nc.tensor.matmul(kp[:], wk_sb[:, t, :], tiles[t][:],
                                     start=(t == 0), stop=(t == DM_TILES - 1))
            if finish is not None:
                finish(0)
                finish(1)

            qt = qt_pool.tile([P, H, FD], bf16, tag="qt", name=f"qt_{qc}")
            qp = [ps.tile([P, FD], f32, tag="ps", name=f"qp{h}_{qc}")
                  for h in range(H)]
            for t in range(DM_TILES):
                if qc == 0:
                    nc.sync.dma_start(wq_sb[:, :, t, :], wq_r[:, :, t, :])
                mm = dict(start=(t == 0), stop=(t == DM_TILES - 1))
                for h in range(H):
                    nc.tensor.matmul(qp[h][:], wq_sb[:, h, t, :], tiles[t][:], **mm)
            if qc == 0:
                nc.sync.dma_start(cos_sb[:], cos2)
                nc.sync.dma_start(sin_sb[:], sin2)
            rotary_evac(kp, kt_sb[:, ds(qc * FD, FD)], bk_sb, qc)
            for h in range(H):
                rotary_evac(qp[h], qt[:, h, :], bq_sb[:, h, :], qc)

            # V: project [d, k], bias on Act, transpose to natural [k, d]
            vp = ps.tile([P, FD], f32, tag="ps", name=f"vp_{qc}")
            for t in range(DM_TILES):
                if qc == 0:
                    nc.sync.dma_start(wv_sb[:, t, :], wv_r[:, t, :])
                nc.tensor.matmul(vp[:], wv_sb[:, t, :], tiles[t][:],
                                 start=(t == 0), stop=(t == DM_TILES - 1))
            if qc == 0:
                nc.sync.dma_start(wo_sb[:], wo.rearrange("h p m -> p h m"))
            vt = sc_pool.tile([P, FD], bf16, tag="vt", name=f"vt_{qc}")
            nc.scalar.activation(vt[:], vp[:], AF.Identity, bias=bv_sb[:, 0:1])
            for j in range(KB):
                tp = ps.tile([P, P], bf16, tag="ps", name=f"tp_{qc}_{j}")
                nc.tensor.transpose(tp[:], vt[:, ds(j * P, P)], id_sb[:])
                nc.vector.tensor_copy(v_sb[:, qc * KB + j, :], tp[:])
            return qt

        def attention(qc, qt):
            """Causal attention for q chunk qc; returns per-head normalized
            z^T (bf16). Inner loop is skewed one full kt iteration: st
            matmuls of both heads first, then PV/den of the previous kt."""
            KT = 4 * qc + 4
            zt = [ps.tile([P, FD], f32, tag="ps", name=f"zt{h}_{qc}")
                  for h in range(H)]
            den = [ps.tile([P, FD], f32, tag="ps", name=f"den{h}_{qc}")
                   for h in range(H)]
            pend = []
            for kt in range(KT):
                o = max(0, kt * P - qc * FD)
                cur = []
                for h in range(H):
                    st = ps.tile([P, FD], f32, tag="ps", name=f"st{h}_{qc}_{kt}")
                    nc.tensor.matmul(st[:, o:FD], kt_sb[:, ds(kt * P, P)],
                                     qt[:, h, o:FD], start=True, stop=True)
                    e = e_pool.tile([P, FD], bf16, tag="e", name=f"e{h}_{qc}_{kt}")
                    nc.scalar.activation(e[:, o:FD], st[:, o:FD], AF.Exp,
                                         scale=1.0 / ATTN_SCALE)
                    if kt >= 4 * qc:  # diagonal 128-block: causal mask inside
                        nc.vector.tensor_mul(e[:, o:o + P], e[:, o:o + P],
                                             mask_sb[:])
                    cur.append((h, kt, e, o))
                for h, pkt, e, po in pend:
                    acc = dict(start=(pkt == 0), stop=(pkt == KT - 1))
                    nc.tensor.matmul(zt[h][:, po:FD], v_sb[:, pkt, :],
                                     e[:, po:FD], **acc)
                    nc.tensor.matmul(den[h][0:1, po:FD], ones_sb[:, 0:1],
                                     e[:, po:FD], **acc)
                pend = cur
            for h, pkt, e, po in pend:
                acc = dict(start=(pkt == 0), stop=(pkt == KT - 1))
                nc.tensor.matmul(zt[h][:, po:FD], v_sb[:, pkt, :],
                                 e[:, po:FD], **acc)
                nc.tensor.matmul(den[h][0:1, po:FD], ones_sb[:, 0:1],
                                 e[:, po:FD], **acc)
            # normalization: reciprocal of one denominator row (DVE, emitted
            # now), then a deferred finish: broadcast via K=1 matmul into the
            # (already-read) den bank (PE, called from the next proj so the
            # PE never waits on the DVE), 1/den copy on Act, z = zt * (1/den)
            rrs = []
            ztn = []
            for h in range(H):
                rf = sc_pool.tile([1, FD], f32, tag="rf", name=f"rf{h}_{qc}")
                nc.vector.reciprocal_approx_fast(rf[:], den[h][0:1, :])
                rr = sc_pool.tile([1, FD], f32r, tag="rr", name=f"rr{h}_{qc}")
                nc.vector.tensor_scalar_mul(rr[:], rf[:], 1.0)
                rrs.append(rr)
                z = sc_pool.tile([P, FD], bf16, tag="z", bufs=4, name=f"z{h}_{qc}")
                ztn.append(z)

            def finish(h):
                nc.tensor.matmul(den[h][:], onesr_sb[:], rrs[h][:],
                                 start=True, stop=True)
                rden = sc_pool.tile([P, FD], f32, tag="rden", name=f"rd{h}_{qc}")
                nc.scalar.copy(rden[:], den[h][:])
                nc.vector.tensor_mul(ztn[h][:], zt[h][:], rden[:])

            return ztn, finish

        def outproj(qc, ztn):
            for sub in range(KB):
                for mc in range(MC):
                    op_ps = ps.tile([P, FD], f32, tag="ps",
                                    name=f"op_{qc}_{sub}_{mc}")
                    for h in range(H):
                        nc.tensor.matmul(op_ps[:], ztn[h][:, ds(sub * P, P)],
                                         wo_sb[:, h, ds(mc * FD, FD)],
                                         start=(h == 0), stop=(h == H - 1))
                    ot = sc_pool.tile([P, FD], bf16, tag="ot", bufs=3,
                                      name=f"ot_{qc}_{sub}_{mc}")
                    nc.vector.tensor_copy(ot[:], op_ps[:])
                    nc.sync.dma_start(out[ds(qc * FD + sub * P, P), ds(mc * FD, FD)],
                                      ot[:])

        qts = {0: proj(0)}
        for qc in range(QC):
            if qc + 1 < QC:
                prefetch_x(qc + 1)
            ztn, finish = attention(qc, qts.pop(qc))
            if qc + 1 < QC:
                qts[qc + 1] = proj(qc + 1, finish)
            else:
                finish(0)
                finish(1)
            outproj(qc, ztn)
    nc.compile()
    return nc


_PERM = None


def _perm():
    global _PERM
    if _PERM is None:
        _PERM = np.concatenate([np.arange(0, D_HEAD, 2), np.arange(1, D_HEAD, 2)])
    return _PERM


def host_inputs(x, W_Q, W_K, W_V, W_O, b_Q, b_K, b_V, core,
                heads_per_core=2):
    """Build the per-core input map (numpy, named as in build_bass)."""
    import ml_dtypes

    bf16 = ml_dtypes.bfloat16
    seq = x.shape[0]
    perm = _perm()
    h0 = core * heads_per_core
    kv = h0 // (N_HEADS // N_KV)
    pairs = D_HEAD // 2
    freqs = 1.0 / ROTARY_BASE ** (np.arange(pairs, dtype=np.float64) / pairs)
    ang = np.outer(np.arange(seq), freqs)  # [seq, 64]
    cos = np.cos(ang).T.astype(np.float32)  # [64, seq]
    sin = np.sin(ang).T.astype(np.float32)
    return {
        "xT": np.ascontiguousarray(np.asarray(x).T.astype(bf16)),
        "wq": np.ascontiguousarray(
            W_Q[h0:h0 + heads_per_core][:, :, perm].astype(bf16)),
        "wk": np.ascontiguousarray(W_K[kv][:, perm].astype(bf16)),
        "wv": np.ascontiguousarray(W_V[kv].astype(bf16)),
        "wo": np.ascontiguousarray(W_O[h0:h0 + heads_per_core].astype(bf16)),
        "bq": np.ascontiguousarray(
            b_Q[h0:h0 + heads_per_core][:, perm]
            .reshape(heads_per_core, 2, 64).transpose(2, 0, 1)
            .astype(np.float32)),
        "bk": np.ascontiguousarray(b_K[kv][perm].reshape(2, 64).T
                                   .astype(np.float32)),
        "bv": np.ascontiguousarray(np.asarray(b_V[kv], np.float32)[:, None]),
        "ident": np.eye(P, dtype=np.float32).astype(bf16),
        "cos2": cos,
        "sin2": sin,
        "maskm": np.triu(np.ones((P, P), dtype=np.float32)).astype(bf16),
        "onesd": np.ones((P, 1), dtype=np.float32).astype(bf16),
        "onesr": np.ones((1, P), dtype=np.float32),
    }


_NC_CACHE = {}


def kernel(x, W_Q, W_K, W_V, W_O, b_Q, b_K, b_V, b_O):
    import sys
    if "/opt/trn_rl_repo" not in sys.path:
        sys.path.insert(0, "/opt/trn_rl_repo")
    from concourse import bass_utils

    x = np.asarray(x, dtype=np.float32)
    key = (x.shape[0], x.shape[1])
    if key not in _NC_CACHE:
        _NC_CACHE[key] = build_bass(seq=x.shape[0], d_model=x.shape[1])
    nc = _NC_CACHE[key]

    in_maps = [
        host_inputs(x, np.asarray(W_Q, np.float32), np.asarray(W_K, np.float32),
                    np.asarray(W_V, np.float32), np.asarray(W_O, np.float32),
                    np.asarray(b_Q, np.float32), np.asarray(b_K, np.float32),
                    np.asarray(b_V, np.float32), core)
        for core in range(N_CORES)
    ]
    res = bass_utils.run_bass_kernel_spmd(nc, in_maps, core_ids=list(range(N_CORES)))
    total = np.zeros((x.shape[0], x.shape[1]), dtype=np.float32)
    for r in res.results:
        total += np.asarray(r["out"], dtype=np.float32)
    total += np.asarray(b_O, np.float32)[None, :]
    return total
